# revision 1
# baseline (speedup 1.0000x reference)
"""Trainium2 Bass kernel for a 2-layer GATv2 + GraphNorm node classifier.

Strategy (8 NeuronCores, SPMD single NEFF):
  - Nodes are sharded contiguously: core k owns nodes [k*6250, (k+1)*6250).
  - Host (index-only preprocessing): add self loops, route each edge to the
    owner of its dst, sort by dst, group into 128-node blocks, pad each block's
    edge list to a whole number of 128-edge tiles (schedule shared by all
    cores so one program serves all), remap src to a padded table index,
    pre-transpose x.
  - Device per core: build the full xl=x@Wl+bl gather table (replicated),
    local xr blocks; per 128-edge tile: one-hot(dst) matrix via is_equal,
    TensorE matmuls for xr broadcast + attention-dot + softmax-weighted
    scatter-add accumulated in PSUM per 128-node block.  GraphNorm is folded
    into the next layer's weights (stats via matmul + AllReduce[64,2]);
    layer-2 gather table built after AllGather of h1 (transposed layout).
  - Softmax max-subtraction is skipped (|e| bounded ~<10, exp is safe in f32).
"""

import numpy as np

import concourse.bacc as bacc
import concourse.bass as bass
import concourse.mybir as mybir
import concourse.tile as tile
from concourse.masks import make_identity

F32 = mybir.dt.float32
I32 = mybir.dt.int32
AF = mybir.ActivationFunctionType
OP = mybir.AluOpType

P = 128


class Cfg:
    def __init__(self, n_nodes, n_cores=8):
        assert n_nodes % n_cores == 0
        self.N = n_nodes
        self.NC = n_cores
        self.NPC = n_nodes // n_cores          # real nodes per core
        self.BLOCKS = (self.NPC + P - 1) // P  # 128-node blocks per core
        self.NPADC = self.BLOCKS * P           # padded nodes per core
        self.NPAD_ALL = self.NC * self.NPADC   # padded table rows
        self.DIN = 128
        self.HC = 128                          # H*C
        self.C = 64
        self.NCLS = 4
        self.EPS = 1e-5


def _preprocess(cfg, x, edge_index):
    """Host-side index preprocessing + input staging. Returns (T_list, in_maps_extra)."""
    N, NC, NPC, BLOCKS, NPADC = cfg.N, cfg.NC, cfg.NPC, cfg.BLOCKS, cfg.NPADC
    E = edge_index.shape[1]
    src = np.concatenate([edge_index[0].astype(np.int64), np.arange(N, dtype=np.int64)])
    dst = np.concatenate([edge_index[1].astype(np.int64), np.arange(N, dtype=np.int64)])

    core = dst // NPC
    dloc = dst - core * NPC
    blk = dloc // P
    dstl = dloc - blk * P                      # within-block dst index [0,128)
    gb = core * BLOCKS + blk                   # global (core, block) id

    # per-(core,block) counts -> shared tile schedule
    cnt = np.bincount(gb, minlength=NC * BLOCKS).reshape(NC, BLOCKS)
    T_list = np.maximum(1, (cnt.max(axis=0) + P - 1) // P).astype(np.int64)  # [BLOCKS]
    T_total = int(T_list.sum())
    offs = np.concatenate([[0], np.cumsum(T_list)])  # tile offset per block

    srcr = (src // NPC) * NPADC + (src % NPC)  # remapped src (padded table row)

    esrcT = np.zeros((NC, P, T_total), dtype=np.int32)
    edstlT = np.full((NC, P, T_total), -1.0, dtype=np.float32)

    order = np.lexsort((dstl, gb))  # sort edges by (core, block) then dstl (any in-block order ok)
    gb_s, dstl_s, srcr_s = gb[order], dstl[order], srcr[order]
    # slot position of each edge within its (core, block) group
    pos_in_group = np.arange(len(gb_s)) - np.searchsorted(gb_s, gb_s, side="left")
    core_s = gb_s // BLOCKS
    blk_s = gb_s % BLOCKS
    slot = offs[blk_s] * P + pos_in_group      # flat slot inside this core's edge array
    tile_i = slot // P
    part_i = slot % P
    esrcT[core_s, part_i, tile_i] = srcr_s
    edstlT[core_s, part_i, tile_i] = dstl_s.astype(np.float32)

    # transposed, padded x
    xT = np.zeros((cfg.DIN, cfg.NPAD_ALL), dtype=np.float32)
    xsrc = np.ascontiguousarray(x.T)  # [DIN, N]
    for k in range(NC):
        xT[:, k * NPADC : k * NPADC + NPC] = xsrc[:, k * NPC : (k + 1) * NPC]

    per_core = []
    for k in range(NC):
        per_core.append({
            "xT": np.ascontiguousarray(xT),
            "xTl": np.ascontiguousarray(xT[:, k * NPADC : (k + 1) * NPADC]),
            "esrcT": np.ascontiguousarray(esrcT[k]),
            "edstlT": np.ascontiguousarray(edstlT[k]),
        })
    return [int(t) for t in T_list], per_core


def _build(cfg, T_list):
    """Build + compile the SPMD program. Returns nc."""
    NC, BLOCKS, NPADC, NPAD_ALL = cfg.NC, cfg.BLOCKS, cfg.NPADC, cfg.NPAD_ALL
    NPC, HC, C, NCLS = cfg.NPC, cfg.HC, cfg.C, cfg.NCLS
    T_total = sum(T_list)
    NT = NC * BLOCKS
    rg = [list(range(NC))]
    LAST = NPC - (BLOCKS - 1) * P  # real rows in last block

    nc = bacc.Bacc("TRN2", target_bir_lowering=False, debug=False,
                   enable_asserts=False, num_devices=NC)

    # ---------------- IO ----------------
    xT_d = nc.dram_tensor("xT", [128, NPAD_ALL], F32, kind="ExternalInput")
    xTl_d = nc.dram_tensor("xTl", [128, NPADC], F32, kind="ExternalInput")
    esrcT_d = nc.dram_tensor("esrcT", [P, T_total], I32, kind="ExternalInput")
    edstlT_d = nc.dram_tensor("edstlT", [P, T_total], F32, kind="ExternalInput")
    w = {}
    for li, din in ((1, 128), (2, 64)):
        w[f"Wl{li}"] = nc.dram_tensor(f"Wl{li}", [din, HC], F32, kind="ExternalInput")
        w[f"Wr{li}"] = nc.dram_tensor(f"Wr{li}", [din, HC], F32, kind="ExternalInput")
        w[f"bl{li}"] = nc.dram_tensor(f"bl{li}", [HC], F32, kind="ExternalInput")
        w[f"br{li}"] = nc.dram_tensor(f"br{li}", [HC], F32, kind="ExternalInput")
        w[f"att{li}"] = nc.dram_tensor(f"att{li}", [2, C], F32, kind="ExternalInput")
        w[f"bias{li}"] = nc.dram_tensor(f"bias{li}", [C], F32, kind="ExternalInput")
        w[f"gng{li}"] = nc.dram_tensor(f"gng{li}", [C], F32, kind="ExternalInput")
        w[f"gnb{li}"] = nc.dram_tensor(f"gnb{li}", [C], F32, kind="ExternalInput")
        w[f"gna{li}"] = nc.dram_tensor(f"gna{li}", [C], F32, kind="ExternalInput")
    W1_d = nc.dram_tensor("W1", [C, NCLS], F32, kind="ExternalInput")
    b1_d = nc.dram_tensor("b1", [NCLS], F32, kind="ExternalInput")
    out_d = nc.dram_tensor("out", [NPC, NCLS], F32, kind="ExternalOutput")
    import os as _os
    DBG = bool(int(_os.environ.get("GAT_DEBUG", "0")))
    if DBG:
        dbg_xl1 = nc.dram_tensor("dbg_xl1", [256, HC], F32, kind="ExternalOutput")
        dbg_h1T = nc.dram_tensor("dbg_h1T", [C, NPADC], F32, kind="ExternalOutput")
        dbg_st = nc.dram_tensor("dbg_st", [C, 2], F32, kind="ExternalOutput")
        dbg_xr1 = nc.dram_tensor("dbg_xr1", [P, HC], F32, kind="ExternalOutput")
        dbg_AB = nc.dram_tensor("dbg_AB", [C, 2], F32, kind="ExternalOutput")
        dbg_ag = nc.dram_tensor("dbg_ag", [C * NC, 128], F32, kind="ExternalOutput")
        dbg_xl2 = nc.dram_tensor("dbg_xl2", [256, HC], F32, kind="ExternalOutput")
        dbg_xr2 = nc.dram_tensor("dbg_xr2", [P, HC], F32, kind="ExternalOutput")
        dbg_h2T = nc.dram_tensor("dbg_h2T", [C, NPADC], F32, kind="ExternalOutput")

    # ---------------- internal DRAM ----------------
    xl1_t = nc.dram_tensor("xl1_t", [NPAD_ALL, HC], F32, kind="Internal")
    xl2_t = nc.dram_tensor("xl2_t", [NPAD_ALL, HC], F32, kind="Internal")
    h1T_dr = nc.dram_tensor("h1T_dr", [C, NPADC], F32, kind="Internal")
    h1T_ag = nc.dram_tensor("h1T_ag", [C * NC, NPADC], F32, kind="Internal", addr_space="Shared")
    st_l = [nc.dram_tensor(f"st{li}_l", [C, 2], F32, kind="Internal") for li in (1, 2)]
    st_g = [nc.dram_tensor(f"st{li}_g", [C, 2], F32, kind="Internal", addr_space="Shared") for li in (1, 2)]

    with tile.TileContext(nc) as tc:
        import contextlib
        ctx = contextlib.ExitStack()
        with ctx:
            con = ctx.enter_context(tc.tile_pool(name="con", bufs=1))
            res = ctx.enter_context(tc.tile_pool(name="res", bufs=1))
            sb = ctx.enter_context(tc.tile_pool(name="sb", bufs=4))
            sgath = ctx.enter_context(tc.tile_pool(name="sgath", bufs=6))
            sidx = ctx.enter_context(tc.tile_pool(name="sidx", bufs=2))
            ps_t = ctx.enter_context(tc.tile_pool(name="ps_t", bufs=1, space="PSUM"))
            ps_b = ctx.enter_context(tc.tile_pool(name="ps_b", bufs=2, space="PSUM"))
            ps_e = ctx.enter_context(tc.tile_pool(name="ps_e", bufs=1, space="PSUM"))
            ps_pet = ctx.enter_context(tc.tile_pool(name="ps_pet", bufs=1, space="PSUM"))
            ps_acc = ctx.enter_context(tc.tile_pool(name="ps_acc", bufs=2, space="PSUM"))
            ps_st = ctx.enter_context(tc.tile_pool(name="ps_st", bufs=1, space="PSUM"))

            # ---------------- constants ----------------
            ident = con.tile([P, P], F32)
            make_identity(nc, ident[:])
            iota_i = con.tile([P, P], I32)
            nc.gpsimd.iota(iota_i[:], pattern=[[1, P]], base=0, channel_multiplier=0)
            iota_f = con.tile([P, P], F32)
            nc.vector.tensor_copy(iota_f[:], iota_i[:])
            ones_col = con.tile([P, 1], F32)
            nc.vector.memset(ones_col[:], 1.0)
            ones_row = con.tile([1, P], F32)
            nc.vector.memset(ones_row[:], 1.0)
            # mask column: 1.0 for real rows of the last node block, 0 for pad
            mask_col = con.tile([P, 1], F32)
            nc.vector.memset(mask_col[:], 1.0)
            if LAST < P:
                nc.gpsimd.affine_select(
                    out=mask_col[:], in_=mask_col[:], compare_op=OP.is_ge,
                    fill=0.0, base=LAST - 1, channel_multiplier=-1, pattern=[[0, 1]])

            def load_row(d, n):  # [n] dram -> [1, n] sbuf
                t = con.tile([1, n], F32, tag=f"row_{d.name}")
                nc.sync.dma_start(out=t[:], in_=d[None, :])
                return t

            def load_col(d, n):  # [n] dram -> [n, 1] sbuf
                t = con.tile([n, 1], F32, tag=f"col_{d.name}")
                nc.sync.dma_start(out=t[:], in_=d[:, None])
                return t

            def replicate_row(row_t, n, tag):  # [1,n] -> [P,n]
                pr = ps_b.tile([P, n], F32, space="PSUM", tag="ps_mm")
                nc.tensor.matmul(pr[:], lhsT=ones_row[:], rhs=row_t[:], start=True, stop=True)
                t = con.tile([P, n], F32, tag=tag)
                nc.scalar.copy(t[:], pr[:])
                return t

            Wl1_sb = con.tile([128, HC], F32)
            nc.sync.dma_start(out=Wl1_sb[:], in_=w["Wl1"][:])
            Wr1_sb = con.tile([128, HC], F32)
            nc.sync.dma_start(out=Wr1_sb[:], in_=w["Wr1"][:])
            Wl2_sb = con.tile([C, HC], F32)
            nc.sync.dma_start(out=Wl2_sb[:], in_=w["Wl2"][:])
            Wr2_sb = con.tile([C, HC], F32)
            nc.sync.dma_start(out=Wr2_sb[:], in_=w["Wr2"][:])
            W1_sb = con.tile([C, NCLS], F32)
            nc.sync.dma_start(out=W1_sb[:], in_=W1_d[:])
            b1_row = load_row(b1_d, NCLS)

            bl1_rep = replicate_row(load_row(w["bl1"], HC), HC, "bl1_rep")
            br1_rep = replicate_row(load_row(w["br1"], HC), HC, "br1_rep")
            bias_rep = [replicate_row(load_row(w[f"bias{li}"], C), C, f"bias{li}_rep") for li in (1, 2)]

            attm = []
            for li in (1, 2):
                t = con.tile([P, 2], F32, tag=f"attm{li}")
                nc.vector.memset(t[:], 0.0)
                nc.sync.dma_start(out=t[0:C, 0:1], in_=w[f"att{li}"][0, :][:, None])
                nc.sync.dma_start(out=t[C:2 * C, 1:2], in_=w[f"att{li}"][1, :][:, None])
                attm.append(t)

            # ---------------- layer-1 tables ----------------
            xr1_res = res.tile([P, BLOCKS, HC], F32, tag="xr1res")
            for j in range(NT):
                xt = sb.tile([128, P], F32, tag="xt")
                nc.sync.dma_start(out=xt[:], in_=xT_d[:, j * P:(j + 1) * P])
                pm = ps_b.tile([P, HC], F32, space="PSUM", tag="ps_mm")
                nc.tensor.matmul(pm[:], lhsT=xt[:], rhs=Wl1_sb[:], start=True, stop=True)
                xlt = sb.tile([P, HC], F32, tag="xlt")
                nc.vector.tensor_add(xlt[:], pm[:], bl1_rep[:])
                nc.sync.dma_start(out=xl1_t[j * P:(j + 1) * P, :], in_=xlt[:])
            for b in range(BLOCKS):
                xt = sb.tile([128, P], F32, tag="xt")
                nc.sync.dma_start(out=xt[:], in_=xTl_d[:, b * P:(b + 1) * P])
                pm = ps_b.tile([P, HC], F32, space="PSUM", tag="ps_mm")
                nc.tensor.matmul(pm[:], lhsT=xt[:], rhs=Wr1_sb[:], start=True, stop=True)
                nc.vector.tensor_add(xr1_res[:, b, :], pm[:], br1_rep[:])

            # ---------------- edge phase (shared for both layers) ----------------
            h1T_res = res.tile([C, NPADC], F32, tag="h1T")
            h2T_res = res.tile([C, NPADC], F32, tag="h2T")

            def edge_layer(li, table, xr_res, hT_res, b_rep):
                pstats = ps_st.tile([C, C + 1], F32, space="PSUM", tag="ps_stats")
                for b in range(BLOCKS):
                    Tb = T_list[b]
                    c0 = sum(T_list[:b])
                    srcg = sidx.tile([P, Tb], I32, tag="srcg")
                    nc.sync.dma_start(out=srcg[:], in_=esrcT_d[:, c0:c0 + Tb])
                    dstg = sidx.tile([P, Tb], F32, tag="dstg")
                    nc.sync.dma_start(out=dstg[:], in_=edstlT_d[:, c0:c0 + Tb])
                    acc = ps_acc.tile([P, HC + 2], F32, space="PSUM", tag="ps_acc")
                    for t in range(Tb):
                        oh = sb.tile([P, P], F32, tag="oh")
                        nc.vector.tensor_tensor(out=oh[:], in0=iota_f[:],
                                                in1=dstg[:, t:t + 1].to_broadcast([P, P]),
                                                op=OP.is_equal)
                        pt = ps_t.tile([P, P], F32, space="PSUM", tag="ps_tr")
                        nc.tensor.transpose(pt[:], oh[:], ident[:])
                        ohT = sb.tile([P, P], F32, tag="ohT")
                        nc.vector.tensor_copy(ohT[:], pt[:])
                        xls = sgath.tile([P, HC], F32, tag="xls")
                        nc.gpsimd.indirect_dma_start(
                            out=xls[:], out_offset=None, in_=table[:],
                            in_offset=bass.IndirectOffsetOnAxis(ap=srcg[:, t:t + 1], axis=0))
                        pb = ps_b.tile([P, P], F32, space="PSUM", tag="ps_mm")
                        nc.tensor.matmul(pb[:], lhsT=xls[:], rhs=ident[:], start=True, stop=False)
                        nc.tensor.matmul(pb[:], lhsT=xr_res[:, b, :], rhs=ohT[:], start=False, stop=True)
                        s02 = sb.tile([P, P], F32, tag="s02")
                        nc.scalar.activation(s02[:], pb[:], AF.Copy, bias=0.0, scale=0.2)
                        r08 = sb.tile([P, P], F32, tag="r08")
                        nc.scalar.activation(r08[:], pb[:], AF.Relu, bias=0.0, scale=0.8)
                        pe = ps_e.tile([2, P], F32, space="PSUM", tag="ps_e")
                        nc.tensor.matmul(pe[:], lhsT=attm[li - 1][:], rhs=s02[:], start=True, stop=False)
                        nc.tensor.matmul(pe[:], lhsT=attm[li - 1][:], rhs=r08[:], start=False, stop=True)
                        eeT = sb.tile([2, P], F32, tag="eeT")
                        nc.scalar.activation(eeT[:], pe[:], AF.Exp)
                        pet = ps_pet.tile([P, 2], F32, space="PSUM", tag="ps_pet")
                        nc.tensor.transpose(pet[:], eeT[:], ident[0:2, 0:2])
                        pay = sb.tile([P, HC + 2], F32, tag="pay")
                        nc.vector.tensor_copy(pay[:, HC:HC + 2], pet[:])
                        nc.vector.tensor_scalar_mul(pay[:, 0:C], xls[:, 0:C], pay[:, HC:HC + 1])
                        nc.vector.tensor_scalar_mul(pay[:, C:HC], xls[:, C:HC], pay[:, HC + 1:HC + 2])
                        nc.tensor.matmul(acc[:], lhsT=oh[:], rhs=pay[:], start=(t == 0), stop=(t == Tb - 1))
                    # ---- drain block b ----
                    last = b == BLOCKS - 1
                    # bias keeps pad-row denominators finite (0 -> 1e-20)
                    d2 = sb.tile([P, 2], F32, tag="d2")
                    nc.scalar.activation(d2[:], acc[:, HC:HC + 2], AF.Copy, bias=1e-20, scale=2.0)
                    rec = sb.tile([P, 2], F32, tag="rec")
                    nc.vector.reciprocal(rec[:], d2[:])
                    t0 = sb.tile([P, C], F32, tag="t0")
                    nc.vector.tensor_scalar_mul(t0[:], acc[:, 0:C], rec[:, 0:1])
                    t1 = sb.tile([P, C], F32, tag="t1")
                    nc.vector.tensor_scalar_mul(t1[:], acc[:, C:HC], rec[:, 1:2])
                    hs = sb.tile([P, C + 1], F32, tag="hs")
                    nc.vector.memset(hs[:, C:C + 1], 1.0)
                    nc.vector.tensor_add(hs[:, 0:C], t0[:], t1[:])
                    hb = hs[:, 0:C]
                    nc.vector.tensor_add(hb, hb, b_rep[:])
                    if last and LAST < P:
                        nc.vector.tensor_scalar_mul(hs[:], hs[:], mask_col[:, 0:1])
                    nc.tensor.matmul(pstats[:], lhsT=hb, rhs=hs[:], start=(b == 0), stop=(b == BLOCKS - 1))
                    pht = ps_t.tile([C, P], F32, space="PSUM", tag="ps_tr")
                    nc.tensor.transpose(pht[:], hb, ident[:])
                    nc.scalar.copy(hT_res[:, b * P:(b + 1) * P], pht[:])
                # ---- stats finalize + AllReduce ----
                trash = sb.tile([C, C], F32, tag="trash")
                st2 = sb.tile([C, 2], F32, tag="st2")
                nc.vector.tensor_mul(trash[:], pstats[:, 0:C], ident[0:C, 0:C])
                nc.vector.tensor_reduce(st2[:, 1:2], trash[:], axis=mybir.AxisListType.X, op=OP.add)
                nc.vector.tensor_copy(st2[:, 0:1], pstats[:, C:C + 1])
                nc.sync.dma_start(out=st_l[li - 1][:], in_=st2[:])
                nc.gpsimd.collective_compute(
                    "AllReduce", OP.add, replica_groups=rg,
                    ins=[st_l[li - 1][:]], outs=[st_g[li - 1][:]])
                stg = sb.tile([C, 2], F32, tag="stg")
                nc.sync.dma_start(out=stg[:], in_=st_g[li - 1][:])
                # A = gng * rsqrt(var+eps); B = gnb - A*a*mean
                a_col = load_col(w[f"gna{li}"], C)
                g_col = load_col(w[f"gng{li}"], C)
                bta_col = load_col(w[f"gnb{li}"], C)
                mean = sb.tile([C, 1], F32, tag="gn_m")
                nc.scalar.activation(mean[:], stg[:, 0:1], AF.Copy, bias=0.0, scale=1.0 / cfg.N)
                msq = sb.tile([C, 1], F32, tag="gn_m2")
                nc.scalar.square(msq[:], mean[:])
                qn = sb.tile([C, 1], F32, tag="gn_qn")
                nc.scalar.activation(qn[:], stg[:, 1:2], AF.Copy, bias=0.0, scale=1.0 / cfg.N)
                a2 = sb.tile([C, 1], F32, tag="gn_a2")
                nc.vector.tensor_mul(a2[:], a_col[:], a_col[:])
                twoa = sb.tile([C, 1], F32, tag="gn_2a")
                nc.scalar.activation(twoa[:], a_col[:], AF.Copy, bias=0.0, scale=2.0)
                coef = sb.tile([C, 1], F32, tag="gn_cf")
                nc.vector.tensor_sub(coef[:], twoa[:], a2[:])
                cm = sb.tile([C, 1], F32, tag="gn_cm")
                nc.vector.tensor_mul(cm[:], coef[:], msq[:])
                var = sb.tile([C, 1], F32, tag="gn_var")
                nc.vector.tensor_sub(var[:], qn[:], cm[:])
                vare = sb.tile([C, 1], F32, tag="gn_vare")
                nc.vector.tensor_scalar_add(vare[:], var[:], cfg.EPS)
                lnv = sb.tile([C, 1], F32, tag="gn_lnv")
                nc.scalar.activation(lnv[:], vare[:], AF.Ln)
                rs = sb.tile([C, 1], F32, tag="gn_rs")
                nc.scalar.activation(rs[:], lnv[:], AF.Exp, bias=0.0, scale=-0.5)
                A = sb.tile([C, 1], F32, tag="gn_A")
                nc.vector.tensor_mul(A[:], g_col[:], rs[:])
                t_ = sb.tile([C, 1], F32, tag="gn_t")
                nc.vector.tensor_mul(t_[:], A[:], a_col[:])
                t2_ = sb.tile([C, 1], F32, tag="gn_t2")
                nc.vector.tensor_mul(t2_[:], t_[:], mean[:])
                B = sb.tile([C, 1], F32, tag="gn_B")
                nc.vector.tensor_sub(B[:], bta_col[:], t2_[:])
                return A, B

            A1, B1 = edge_layer(1, xl1_t, xr1_res, h1T_res, bias_rep[0])

            if DBG:
                nc.sync.dma_start(out=dbg_xl1[:], in_=xl1_t[0:256, :])
                nc.sync.dma_start(out=dbg_h1T[:], in_=h1T_res[:])
                nc.sync.dma_start(out=dbg_st[:], in_=st_g[0][:])
                nc.sync.dma_start(out=dbg_xr1[:], in_=xr1_res[:, 3, :])

            # AllGather h1 (transposed layout)
            nc.sync.dma_start(out=h1T_dr[:], in_=h1T_res[:])
            nc.gpsimd.collective_compute(
                "AllGather", OP.bypass, replica_groups=rg,
                ins=[h1T_dr[:]], outs=[h1T_ag[:]])

            # folded layer-2 weights
            def fold(W_sb, b_d, A, B, ncols, tag):
                Wp = con.tile([C, ncols], F32, tag=f"W_{tag}")
                nc.vector.tensor_scalar_mul(Wp[:], W_sb[:], A[:])
                pbias = ps_b.tile([1, ncols], F32, space="PSUM", tag="ps_mm")
                nc.tensor.matmul(pbias[:], lhsT=B[:], rhs=W_sb[:], start=True, stop=True)
                brow = con.tile([1, ncols], F32, tag=f"brow_{tag}")
                nc.vector.tensor_add(brow[:], pbias[:], load_row(b_d, ncols)[:])
                rep = replicate_row(brow, ncols, f"brep_{tag}")
                return Wp, rep

            Wl2p, bl2p_rep = fold(Wl2_sb, w["bl2"], A1, B1, HC, "l2l")
            Wr2p, br2p_rep = fold(Wr2_sb, w["br2"], A1, B1, HC, "l2r")

            # ---------------- layer-2 tables ----------------
            xr2_res = res.tile([P, BLOCKS, HC], F32, tag="xr2res")
            for j in range(NT):
                k, b = divmod(j, BLOCKS)
                ht = sb.tile([C, P], F32, tag="ht")
                nc.sync.dma_start(out=ht[:], in_=h1T_ag[k * C:(k + 1) * C, b * P:(b + 1) * P])
                pm = ps_b.tile([P, HC], F32, space="PSUM", tag="ps_mm")
                nc.tensor.matmul(pm[:], lhsT=ht[:], rhs=Wl2p[:], start=True, stop=True)
                xlt = sb.tile([P, HC], F32, tag="xlt")
                nc.vector.tensor_add(xlt[:], pm[:], bl2p_rep[:])
                nc.sync.dma_start(out=xl2_t[j * P:(j + 1) * P, :], in_=xlt[:])
            for b in range(BLOCKS):
                pm = ps_b.tile([P, HC], F32, space="PSUM", tag="ps_mm")
                nc.tensor.matmul(pm[:], lhsT=h1T_res[:, b * P:(b + 1) * P], rhs=Wr2p[:], start=True, stop=True)
                nc.vector.tensor_add(xr2_res[:, b, :], pm[:], br2p_rep[:])

            if DBG:
                nc.sync.dma_start(out=dbg_ag[:], in_=h1T_ag[:, 384:512])
                nc.sync.dma_start(out=dbg_xl2[:], in_=xl2_t[0:256, :])
                nc.sync.dma_start(out=dbg_xr2[:], in_=xr2_res[:, 3, :])
                ab = sb.tile([C, 2], F32, tag="dbgab")
                nc.vector.tensor_copy(ab[:, 0:1], A1[:])
                nc.vector.tensor_copy(ab[:, 1:2], B1[:])
                nc.sync.dma_start(out=dbg_AB[:], in_=ab[:])

            A2, B2 = edge_layer(2, xl2_t, xr2_res, h2T_res, bias_rep[1])

            if DBG:
                nc.sync.dma_start(out=dbg_h2T[:], in_=h2T_res[:])

            # ---------------- classifier + log_softmax ----------------
            W1p = con.tile([C, NCLS], F32, tag="W1p")
            nc.vector.tensor_scalar_mul(W1p[:], W1_sb[:], A2[:])
            pb1 = ps_b.tile([1, NCLS], F32, space="PSUM", tag="ps_mm")
            nc.tensor.matmul(pb1[:], lhsT=B2[:], rhs=W1_sb[:], start=True, stop=True)
            b1p = con.tile([1, NCLS], F32, tag="b1p")
            nc.vector.tensor_add(b1p[:], pb1[:], b1_row[:])
            b1p_rep = replicate_row(b1p, NCLS, "b1p_rep")

            for b in range(BLOCKS):
                pl = ps_acc.tile([P, NCLS], F32, space="PSUM", tag="ps_acc")
                nc.tensor.matmul(pl[:], lhsT=h2T_res[:, b * P:(b + 1) * P], rhs=W1p[:], start=True, stop=True)
                lg = sb.tile([P, NCLS], F32, tag="lg")
                nc.vector.tensor_add(lg[:], pl[:], b1p_rep[:])
                mx = sb.tile([P, 1], F32, tag="mx")
                nc.vector.tensor_reduce(mx[:], lg[:], axis=mybir.AxisListType.X, op=OP.max)
                lgm = sb.tile([P, NCLS], F32, tag="lgm")
                nc.vector.tensor_scalar(out=lgm[:], in0=lg[:], scalar1=mx[:, 0:1], scalar2=None, op0=OP.subtract)
                ex = sb.tile([P, NCLS], F32, tag="ex")
                nc.scalar.activation(ex[:], lgm[:], AF.Exp)
                sm = sb.tile([P, 1], F32, tag="sm")
                nc.vector.tensor_reduce(sm[:], ex[:], axis=mybir.AxisListType.X, op=OP.add)
                lns = sb.tile([P, 1], F32, tag="lns")
                nc.scalar.activation(lns[:], sm[:], AF.Ln)
                ot = sb.tile([P, NCLS], F32, tag="ot")
                nc.vector.tensor_scalar(out=ot[:], in0=lgm[:], scalar1=lns[:, 0:1], scalar2=None, op0=OP.subtract)
                rows = min(P, NPC - b * P)
                nc.sync.dma_start(out=out_d[b * P: b * P + rows, :], in_=ot[0:rows, :])

    nc.compile()
    return nc


_CACHE = {}


def _get_program(cfg, T_list):
    key = tuple(T_list)
    if key not in _CACHE:
        _CACHE[key] = _build(cfg, T_list)
    return _CACHE[key]


def _install_axon_ntff_shim():
    """Provide antenv.axon_hooks (missing on this image) so trace=True works
    under axon. Mirrors trn_agent_boot's ctypes hook against libaxon_pjrt.so."""
    import sys, types, ctypes, contextlib, glob as _glob
    try:
        import antenv.axon_hooks  # noqa
        return
    except ImportError:
        pass
    hook = None
    for so_path in (["/opt/axon/libaxon_pjrt.so"] + _glob.glob("/root/.axon_site/**/libaxon_pjrt.so", recursive=True)):
        try:
            lib = ctypes.CDLL(so_path)
        except OSError:
            continue
        if not hasattr(lib, "axon_start_nrt_profile"):
            continue
        lib.axon_start_nrt_profile.argtypes = [ctypes.POINTER(ctypes.c_int64), ctypes.c_size_t]
        lib.axon_start_nrt_profile.restype = ctypes.c_int64
        lib.axon_stop_nrt_profile.argtypes = [ctypes.c_char_p]
        lib.axon_stop_nrt_profile.restype = ctypes.c_int64

        @contextlib.contextmanager
        def _hook(output_dir, device_ids, _lib=lib):
            import jax
            jax.devices()
            if device_ids:
                ids = (ctypes.c_int64 * len(device_ids))(*device_ids)
                rc = _lib.axon_start_nrt_profile(ids, len(device_ids))
            else:
                rc = _lib.axon_start_nrt_profile(None, 0)
            if rc != 0:
                raise RuntimeError(f"axon_start_nrt_profile rc={rc}")
            try:
                yield
            finally:
                n = _lib.axon_stop_nrt_profile(str(output_dir).encode())
                print(f"ntff profile: {n} file(s) -> {output_dir}")

        hook = _hook
        break
    m = types.ModuleType("antenv.axon_hooks")
    m.get_axon_ntff_profile_hook = lambda: hook
    m.set_axon_ntff_profile_hook = lambda h: None
    sys.modules["antenv.axon_hooks"] = m
    try:
        import antenv
        antenv.axon_hooks = m
    except ImportError:
        pass
    # artifact upload has no bucket in this container; keep traces local
    import concourse.bass_utils as bu
    bu.upload_artifacts = lambda tmpdir: str(tmpdir)


def kernel(**inputs):
    from concourse.bass_utils import run_bass_kernel_spmd
    import os

    x = np.ascontiguousarray(np.asarray(inputs["x"], dtype=np.float32))
    edge_index = np.asarray(inputs["edge_index"], dtype=np.int32)
    cfg = Cfg(x.shape[0], 8)
    T_list, per_core = _preprocess(cfg, x, edge_index)
    nc = _get_program(cfg, T_list)

    wnames = []
    for li in (1, 2):
        wnames += [f"Wl{li}", f"bl{li}", f"Wr{li}", f"br{li}", f"att{li}",
                   f"bias{li}", f"gng{li}", f"gnb{li}", f"gna{li}"]
    wnames += ["W1", "b1"]
    base = {}
    for n in wnames:
        a = np.ascontiguousarray(np.asarray(inputs[n], dtype=np.float32))
        if n.startswith(("bl", "br", "bias", "gng", "gnb", "gna", "b1")):
            a = a.reshape(-1)
        base[n] = a
    in_maps = [{**base, **pc} for pc in per_core]

    trace = bool(int(os.environ.get("GAT_TRACE", "0")))
    if trace:
        _install_axon_ntff_shim()
    r = run_bass_kernel_spmd(nc, in_maps, core_ids=list(range(cfg.NC)), trace=trace)
    kernel.last_results = r
    if trace and r.exec_time_ns is not None:
        print(f"HW exec time: {r.exec_time_ns} ns")
        if r.instructions_and_trace is not None:
            print(f"trace: {r.instructions_and_trace[1]}")
        print(f"profile_json: {r.profile_json}")
        kernel.last_exec_ns = r.exec_time_ns
    out = np.concatenate([r.results[k]["out"] for k in range(cfg.NC)], axis=0)
    return out



# revision 5
# speedup vs baseline: 2.2573x; 2.2573x over previous
"""Trainium2 Bass kernel for a 2-layer GATv2 + GraphNorm node classifier.

Strategy (8 NeuronCores, SPMD single NEFF):
  - Nodes sharded contiguously: core k owns nodes [k*6250, (k+1)*6250).
  - Host: add self loops, route edges to dst owner, group into 128-node
    blocks with a shared (max-over-cores) tile schedule, split each block's
    edges by src-table half (row < 32768) so HW dma_gather int16 indices
    stay in range, build wrapped int16 index arrays for the gathers.
  - Device per core, per layer:
      * build fp16 gather tables in DRAM: xl = x@Wl (no bias; bias folded
        into the xr table for e-scores and into the drain for the payload),
        xr = x@Wr + (bl+br).
      * per group of blocks: 3 HW dma_gathers (xl-lo, xl-hi, xr) pull
        per-edge rows into SBUF fp16; one fused DVE chain computes
        leakyrelu(xl[src]+xr[dst]) * att, segmented-reduces per-head
        e-scores, Act engine exponentiates; DVE forms the weighted payload
        [ee*xl | ee]; one fp16 TensorE matmul per 128-edge tile
        scatter-adds into the per-block PSUM accumulator via the dst
        one-hot matrix.
      * drain per block in fp32: normalize per head, mean heads, + bias;
        GraphNorm folded into next layer's weights (stats via matmul +
        AllReduce); h1 AllGathered in fp16 for layer-2 tables.
  - Classifier + log_softmax in fp32 per block.
"""

import numpy as np

import concourse.bacc as bacc
import concourse.bass as bass
import concourse.mybir as mybir
import concourse.tile as tile
from concourse.masks import make_identity
from concourse.library_config import mlp as MLP_LIB

F32 = mybir.dt.float32
F16 = mybir.dt.float16
I16 = mybir.dt.int16
I32 = mybir.dt.int32
AF = mybir.ActivationFunctionType
OP = mybir.AluOpType

P = 128
SPLIT = 32768          # xl table lo/hi row split (int16 index range)
TG_MAX = 40            # max tiles per gather group


class Cfg:
    def __init__(self, n_nodes, n_cores=8):
        assert n_nodes % n_cores == 0
        self.N = n_nodes
        self.NC = n_cores
        self.NPC = n_nodes // n_cores
        self.BLOCKS = (self.NPC + P - 1) // P
        self.NPADC = self.BLOCKS * P
        self.NPAD_ALL = self.NC * self.NPADC
        self.HI_ROWS = self.NPAD_ALL - SPLIT
        self.DIN = 128
        self.HC = 128
        self.C = 64
        self.NCLS = 4
        self.EPS = 1e-5


def _wrap_idx(idx_flat):
    """[n*128] int array -> [128, n*8] int16 wrapped (16-partition) + replicated x8."""
    n = idx_flat.shape[0]
    assert n % 16 == 0
    blk = np.zeros((16, n // 16), np.int16)
    j = np.arange(n)
    blk[j % 16, j // 16] = idx_flat.astype(np.int16)
    return np.tile(blk, (8, 1))


def _preprocess(cfg, x, edge_index):
    """Host-side index preprocessing + input staging."""
    N, NC, NPC, BLOCKS, NPADC = cfg.N, cfg.NC, cfg.NPC, cfg.BLOCKS, cfg.NPADC
    src = np.concatenate([edge_index[0].astype(np.int64), np.arange(N, dtype=np.int64)])
    dst = np.concatenate([edge_index[1].astype(np.int64), np.arange(N, dtype=np.int64)])

    srcr = (src // NPC) * NPADC + (src % NPC)   # padded table row
    core = dst // NPC
    dloc = dst - core * NPC
    blk = dloc // P
    dstl = dloc - blk * P
    lo = srcr < SPLIT

    # per (core, block, half) counts -> shared tile schedule
    gb = core * BLOCKS + blk
    cnt_lo = np.bincount(gb[lo], minlength=NC * BLOCKS).reshape(NC, BLOCKS)
    cnt_hi = np.bincount(gb[~lo], minlength=NC * BLOCKS).reshape(NC, BLOCKS)
    T_lo = ((cnt_lo.max(axis=0) + P - 1) // P).astype(np.int64)
    T_hi = ((cnt_hi.max(axis=0) + P - 1) // P).astype(np.int64)

    # groups of consecutive blocks, sum of tiles <= TG_MAX
    groups = []  # (b0, nb)
    b0 = 0
    while b0 < BLOCKS:
        nb = 0
        tot = 0
        while b0 + nb < BLOCKS and (nb == 0 or tot + T_lo[b0 + nb] + T_hi[b0 + nb] <= TG_MAX):
            tot += T_lo[b0 + nb] + T_hi[b0 + nb]
            nb += 1
        groups.append((b0, nb))
        b0 += nb

    # column layout: per group: [lo tiles of b0..][lo of b0+1..] then [hi tiles..]
    lo_col = np.zeros(BLOCKS, np.int64)   # column of block's first lo tile
    hi_col = np.zeros(BLOCKS, np.int64)
    grp_meta = []  # (col0, T_lo_g, T_hi_g, [blocks])
    c = 0
    for (b0, nb) in groups:
        tlo = int(T_lo[b0:b0 + nb].sum())
        thi = int(T_hi[b0:b0 + nb].sum())
        cc = c
        for b in range(b0, b0 + nb):
            lo_col[b] = cc
            cc += T_lo[b]
        for b in range(b0, b0 + nb):
            hi_col[b] = cc
            cc += T_hi[b]
        grp_meta.append((c, tlo, thi, list(range(b0, b0 + nb))))
        c += tlo + thi
    T_TOT = c
    T_LO_TOT = int(T_lo.sum())
    T_HI_TOT = int(T_hi.sum())

    # per-core slot assignment
    edstl = np.full((NC, P, T_TOT), -1.0, np.float16)
    src_idx = np.zeros((NC, P, T_TOT), np.int64)    # row in (lo|hi) table, pad->0
    xr_idx = np.zeros((NC, P, T_TOT), np.int64)     # row in xr table, pad->0

    order = np.lexsort((~lo, gb))   # group by (core, block), lo first
    gb_s = gb[order]
    lo_s = lo[order]
    srcr_s = srcr[order]
    dstl_s = dstl[order]
    blk_s = blk[order]
    core_s = core[order]
    # position within (core, block, half) run
    key = gb_s * 2 + (~lo_s).astype(np.int64)
    pos = np.arange(len(key)) - np.searchsorted(key, key, side="left")
    colbase = np.where(lo_s, lo_col[blk_s], hi_col[blk_s])
    col = colbase + pos // P
    part = pos % P
    srow = np.where(lo_s, srcr_s, srcr_s - SPLIT)
    edstl[core_s, part, col] = dstl_s.astype(np.float16)
    src_idx[core_s, part, col] = srow
    xr_idx[core_s, part, col] = blk_s * P + dstl_s

    # wrapped idx arrays per gather call, concatenated along columns
    def build_idx(core_k):
        lo_parts, hi_parts, xr_parts = [], [], []
        for (c0, tlo, thi, _bs) in grp_meta:
            if tlo:
                cols = src_idx[core_k][:, c0:c0 + tlo]       # [P, tlo]
                lo_parts.append(_wrap_idx(cols.T.reshape(-1)))
            if thi:
                cols = src_idx[core_k][:, c0 + tlo:c0 + tlo + thi]
                hi_parts.append(_wrap_idx(cols.T.reshape(-1)))
            cols = xr_idx[core_k][:, c0:c0 + tlo + thi]
            xr_parts.append(_wrap_idx(cols.T.reshape(-1)))
        cat = lambda ps: (np.concatenate(ps, axis=1) if ps
                          else np.zeros((128, 0), np.int16))
        return cat(lo_parts), cat(hi_parts), cat(xr_parts)

    # transposed, padded x (fp16)
    xT = np.zeros((cfg.DIN, cfg.NPAD_ALL), dtype=np.float16)
    xsrc = np.ascontiguousarray(x.T)
    for k in range(NC):
        xT[:, k * NPADC: k * NPADC + NPC] = xsrc[:, k * NPC:(k + 1) * NPC]

    per_core = []
    for k in range(NC):
        ilo, ihi, ixr = build_idx(k)
        per_core.append({
            "xT": xT,
            "xTl": np.ascontiguousarray(xT[:, k * NPADC:(k + 1) * NPADC]),
            "idx_lo": np.ascontiguousarray(ilo),
            "idx_hi": np.ascontiguousarray(ihi),
            "idx_xr": np.ascontiguousarray(ixr),
            "edstl": np.ascontiguousarray(edstl[k]),
        })

    assert T_LO_TOT > 0 and T_HI_TOT > 0
    sched = {
        "T_lo": [int(v) for v in T_lo], "T_hi": [int(v) for v in T_hi],
        "lo_col": [int(v) for v in lo_col], "hi_col": [int(v) for v in hi_col],
        "grp_meta": [(int(a), int(b), int(cc), list(map(int, d))) for (a, b, cc, d) in grp_meta],
        "T_TOT": int(T_TOT), "T_LO_TOT": T_LO_TOT, "T_HI_TOT": T_HI_TOT,
    }
    return sched, per_core


def _build(cfg, sched):
    NC, BLOCKS, NPADC, NPAD_ALL = cfg.NC, cfg.BLOCKS, cfg.NPADC, cfg.NPAD_ALL
    NPC, HC, C, NCLS = cfg.NPC, cfg.HC, cfg.C, cfg.NCLS
    T_lo, T_hi = sched["T_lo"], sched["T_hi"]
    lo_col, hi_col = sched["lo_col"], sched["hi_col"]
    grp_meta = sched["grp_meta"]
    T_TOT, T_LO_TOT, T_HI_TOT = sched["T_TOT"], sched["T_LO_TOT"], sched["T_HI_TOT"]
    NT = NC * BLOCKS                      # 392 node tiles
    NT_LO = SPLIT // P                    # 256 node tiles in lo table
    rg = [list(range(NC))]
    LAST = NPC - (BLOCKS - 1) * P
    CHUNK = 8                             # node tiles per load/store DMA

    nc = bacc.Bacc("TRN2", target_bir_lowering=False, debug=False,
                   enable_asserts=False, num_devices=NC, num_swdge_queues=4)

    # ---------------- IO ----------------
    xT_d = nc.dram_tensor("xT", [128, NPAD_ALL], F16, kind="ExternalInput")
    xTl_d = nc.dram_tensor("xTl", [128, NPADC], F16, kind="ExternalInput")
    idx_lo_d = nc.dram_tensor("idx_lo", [128, max(T_LO_TOT * 8, 16)], I16, kind="ExternalInput")
    idx_hi_d = nc.dram_tensor("idx_hi", [128, max(T_HI_TOT * 8, 16)], I16, kind="ExternalInput")
    idx_xr_d = nc.dram_tensor("idx_xr", [128, T_TOT * 8], I16, kind="ExternalInput")
    edstl_d = nc.dram_tensor("edstl", [P, T_TOT], F16, kind="ExternalInput")
    Wl1h_d = nc.dram_tensor("Wl1h", [128, HC], F16, kind="ExternalInput")
    Wr1h_d = nc.dram_tensor("Wr1h", [128, HC], F16, kind="ExternalInput")
    cb1_rep_d = nc.dram_tensor("cb1_rep", [P, HC], F32, kind="ExternalInput")
    B1_rep_d = nc.dram_tensor("B1_rep", [P, C], F32, kind="ExternalInput")
    att1_rep_d = nc.dram_tensor("att1_rep", [P, HC], F16, kind="ExternalInput")
    att2_rep_d = nc.dram_tensor("att2_rep", [P, HC], F16, kind="ExternalInput")
    mask_d = nc.dram_tensor("maskcol", [P, 1], F32, kind="ExternalInput")
    Wl2_d = nc.dram_tensor("Wl2", [C, HC], F32, kind="ExternalInput")
    Wr2_d = nc.dram_tensor("Wr2", [C, HC], F32, kind="ExternalInput")
    bl2_d = nc.dram_tensor("bl2", [HC], F32, kind="ExternalInput")
    br2_d = nc.dram_tensor("br2", [HC], F32, kind="ExternalInput")
    bias2_d = nc.dram_tensor("bias2", [C], F32, kind="ExternalInput")
    w = {}
    for li in (1, 2):
        for nm in ("gng", "gnb", "gna"):
            w[f"{nm}{li}"] = nc.dram_tensor(f"{nm}{li}", [C], F32, kind="ExternalInput")
    W1_d = nc.dram_tensor("W1", [C, NCLS], F32, kind="ExternalInput")
    b1_d = nc.dram_tensor("b1", [NCLS], F32, kind="ExternalInput")
    out_d = nc.dram_tensor("out", [NPC, NCLS], F32, kind="ExternalOutput")

    # ---------------- internal DRAM ----------------
    xl_lo_t = [nc.dram_tensor(f"xl{li}_lo", [SPLIT, HC], F16, kind="Internal") for li in (1, 2)]
    xl_hi_t = [nc.dram_tensor(f"xl{li}_hi", [cfg.HI_ROWS, HC], F16, kind="Internal") for li in (1, 2)]
    xr_t = [nc.dram_tensor(f"xr{li}_t", [NPADC, HC], F16, kind="Internal") for li in (1, 2)]
    h1T_dr = nc.dram_tensor("h1T_dr", [C, NPADC], F16, kind="Internal")
    h1T_ag = nc.dram_tensor("h1T_ag", [C * NC, NPADC], F16, kind="Internal", addr_space="Shared")
    st_l = [nc.dram_tensor(f"st{li}_l", [C, 2], F32, kind="Internal") for li in (1, 2)]
    st_g = [nc.dram_tensor(f"st{li}_g", [C, 2], F32, kind="Internal", addr_space="Shared") for li in (1, 2)]

    with tile.TileContext(nc) as tc:
        import contextlib
        ctx = contextlib.ExitStack()
        with ctx:
            con = ctx.enter_context(tc.tile_pool(name="con", bufs=1))
            res = ctx.enter_context(tc.tile_pool(name="res", bufs=1))
            sb = ctx.enter_context(tc.tile_pool(name="sb", bufs=3))
            stg = ctx.enter_context(tc.tile_pool(name="stg", bufs=3))
            gp = ctx.enter_context(tc.tile_pool(name="gp", bufs=2))
            ep = ctx.enter_context(tc.tile_pool(name="ep", bufs=2))
            ps_mm = ctx.enter_context(tc.tile_pool(name="ps_mm", bufs=3, space="PSUM"))
            ps_acc = ctx.enter_context(tc.tile_pool(name="ps_acc", bufs=2, space="PSUM"))
            ps_st = ctx.enter_context(tc.tile_pool(name="ps_st", bufs=1, space="PSUM"))
            ps_t = ctx.enter_context(tc.tile_pool(name="ps_t", bufs=2, space="PSUM"))

            # ---------------- constants (standard gpsimd lib first) ----------------
            ident = con.tile([P, P], F32)
            make_identity(nc, ident[:])
            iota_i = con.tile([P, P], I32)
            nc.gpsimd.iota(iota_i[:], pattern=[[1, P]], base=0, channel_multiplier=0)
            iota_h = con.tile([P, P], F16)
            nc.vector.tensor_copy(iota_h[:], iota_i[:])
            nc.gpsimd.load_library(MLP_LIB)

            ones_row = con.tile([1, P], F32)
            nc.vector.memset(ones_row[:], 1.0)
            tiny_col = con.tile([P, 1], F32)
            nc.vector.memset(tiny_col[:], 1e-20)

            def cload(d, shape, dt=F32, tag=None):
                t = con.tile(shape, dt, tag=tag or f"c_{d.name}")
                nc.sync.dma_start(out=t[:], in_=d[:])
                return t

            mask_col = cload(mask_d, [P, 1])
            Wl1h = cload(Wl1h_d, [128, HC], F16)
            Wr1h = cload(Wr1h_d, [128, HC], F16)
            cb1_rep = cload(cb1_rep_d, [P, HC])
            B1_rep = cload(B1_rep_d, [P, C])
            att_rep = [cload(att1_rep_d, [P, HC], F16), cload(att2_rep_d, [P, HC], F16)]
            Wl2_sb = cload(Wl2_d, [C, HC])
            Wr2_sb = cload(Wr2_d, [C, HC])
            W1_sb = cload(W1_d, [C, NCLS])

            def load_row(d, n):
                t = con.tile([1, n], F32, tag=f"row_{d.name}")
                nc.sync.dma_start(out=t[:], in_=d[None, :])
                return t

            def load_col(d, n):
                t = con.tile([n, 1], F32, tag=f"col_{d.name}")
                nc.sync.dma_start(out=t[:], in_=d[:, None])
                return t

            bl2_row = load_row(bl2_d, HC)
            br2_row = load_row(br2_d, HC)
            bias2_row = load_row(bias2_d, C)
            b1_row = load_row(b1_d, NCLS)

            def replicate_row(row_ap, n, tag):
                pr = ps_mm.tile([P, n], F32, space="PSUM", tag="ps_tab")
                nc.tensor.matmul(pr[:], lhsT=ones_row[:], rhs=row_ap, start=True, stop=True)
                t = con.tile([P, n], F32, tag=tag)
                nc.scalar.copy(t[:], pr[:])
                return t

            # index arrays + edge dst, resident
            idx_lo_sb = res.tile([128, max(T_LO_TOT * 8, 16)], I16, tag="idxlo")
            nc.sync.dma_start(out=idx_lo_sb[:], in_=idx_lo_d[:])
            idx_hi_sb = res.tile([128, max(T_HI_TOT * 8, 16)], I16, tag="idxhi")
            nc.sync.dma_start(out=idx_hi_sb[:], in_=idx_hi_d[:])
            idx_xr_sb = res.tile([128, T_TOT * 8], I16, tag="idxxr")
            nc.sync.dma_start(out=idx_xr_sb[:], in_=idx_xr_d[:])
            edstl_sb = res.tile([P, T_TOT], F16, tag="edstl")
            nc.sync.dma_start(out=edstl_sb[:], in_=edstl_d[:])

            h1T_res = res.tile([C, NPADC], F16, tag="h1T")
            h2T_res = res.tile([C, NPADC], F16, tag="h2T")

            # ---------------- layer-1 tables ----------------
            def build_xl_table(li, get_lhsT, rhs_f16):
                """xl table (no bias): psum = lhsT@rhs, convert fp16, store."""
                n_chunks = NT // CHUNK
                for ch in range(n_chunks):
                    lhs_tiles = get_lhsT(ch)
                    stage = stg.tile([P, CHUNK, HC], F16, tag="xl_stage")
                    for j in range(CHUNK):
                        pm = ps_mm.tile([P, HC], F32, space="PSUM", tag="ps_tab")
                        nc.tensor.matmul(pm[:], lhsT=lhs_tiles[j], rhs=rhs_f16[:],
                                         start=True, stop=True)
                        if j % 2 == 0:
                            nc.scalar.copy(stage[:, j, :], pm[:])
                        else:
                            nc.vector.tensor_copy(stage[:, j, :], pm[:])
                    t0 = ch * CHUNK
                    if (t0 + CHUNK) * P <= SPLIT:
                        dst = xl_lo_t[li - 1][t0 * P:(t0 + CHUNK) * P, :]
                    else:
                        dst = xl_hi_t[li - 1][t0 * P - SPLIT:(t0 + CHUNK) * P - SPLIT, :]
                    nc.sync.dma_start(
                        out=dst.rearrange("(t p) c -> p t c", p=P),
                        in_=stage[:])

            def build_xr_table(li, get_lhsT, rhs_f16, cb_rep):
                for ch in range(BLOCKS // CHUNK + (1 if BLOCKS % CHUNK else 0)):
                    j0 = ch * CHUNK
                    jn = min(CHUNK, BLOCKS - j0)
                    stage = stg.tile([P, CHUNK, HC], F16, tag="xr_stage")
                    for j in range(jn):
                        pm = ps_mm.tile([P, HC], F32, space="PSUM", tag="ps_tab")
                        nc.tensor.matmul(pm[:], lhsT=get_lhsT(j0 + j), rhs=rhs_f16[:],
                                         start=True, stop=True)
                        nc.vector.tensor_add(stage[:, j, :], pm[:], cb_rep[:])
                    nc.sync.dma_start(
                        out=xr_t[li - 1][j0 * P:(j0 + jn) * P, :].rearrange("(t p) c -> p t c", p=P),
                        in_=stage[:, 0:jn, :])

            # layer 1: lhsT chunks from xT (DMA in chunks of 8 tiles)
            xT_chunks = {}

            def get_xt_chunk(ch):
                t = sb.tile([128, CHUNK * P], F16, tag="xt_chunk")
                nc.sync.dma_start(out=t[:], in_=xT_d[:, ch * CHUNK * P:(ch + 1) * CHUNK * P])
                return [t[:, j * P:(j + 1) * P] for j in range(CHUNK)]

            build_xl_table(1, get_xt_chunk, Wl1h)

            xtl_chunks = {}

            def get_xtl(b):
                ch = b // CHUNK
                if ch not in xtl_chunks:
                    n = min(CHUNK * P, NPADC - ch * CHUNK * P)
                    t = sb.tile([128, CHUNK * P], F16, tag="xtl_chunk")
                    nc.sync.dma_start(out=t[:, 0:n], in_=xTl_d[:, ch * CHUNK * P:ch * CHUNK * P + n])
                    xtl_chunks.clear()
                    xtl_chunks[ch] = t
                j = b - ch * CHUNK
                return xtl_chunks[ch][:, j * P:(j + 1) * P]

            build_xr_table(1, get_xtl, Wr1h, cb1_rep)

            # ---------------- edge phase ----------------
            def edge_layer(li, att_rep_t, B_rep, hT_res):
                pstats = ps_st.tile([C, C + 1], F32, space="PSUM", tag="ps_stats")
                n_groups = len(grp_meta)
                lo_used = 0
                hi_used = 0
                GCAP = 8
                qctr = [li * 2]

                def gathers(out_tile, col_off, n_tiles, table, idx_sb, idx_col0):
                    o = 0
                    while o < n_tiles:
                        n = min(GCAP, n_tiles - o)
                        nc.gpsimd.dma_gather(
                            out_ap=out_tile[:, col_off + o:col_off + o + n, :],
                            in_ap=table[:],
                            idxs_ap=idx_sb[:, (idx_col0 + o) * 8:(idx_col0 + o + n) * 8],
                            num_idxs=n * P, num_idxs_reg=n * P, elem_size=HC,
                            queue_num=qctr[0] % 4)
                        qctr[0] += 1
                        o += n

                for gi, (c0, tlo, thi, bs) in enumerate(grp_meta):
                    Tg = tlo + thi
                    xls = gp.tile([P, TG_MAX, HC], F16, tag="xls")
                    gathers(xls, 0, tlo, xl_lo_t[li - 1], idx_lo_sb, lo_used)
                    gathers(xls, tlo, thi, xl_hi_t[li - 1], idx_hi_sb, hi_used)
                    lo_used += tlo
                    hi_used += thi
                    xrd = gp.tile([P, TG_MAX, HC], F16, tag="xrd")
                    gathers(xrd, 0, Tg, xr_t[li - 1], idx_xr_sb, c0)

                    sfl = xrd[:, 0:Tg, :].rearrange("p t c -> p (t c)")
                    xlsf = xls[:, 0:Tg, :].rearrange("p t c -> p (t c)")
                    # s = xl[src] + xr[dst]   (in-place into xrd)
                    nc.vector.tensor_add(sfl, xlsf, sfl)
                    # leaky relu in place: s = max(0.2*s, s)
                    nc.vector.scalar_tensor_tensor(
                        out=sfl, in0=sfl, scalar=0.2, in1=sfl, op0=OP.mult, op1=OP.max)
                    # w = lr * att (in place)
                    s4 = xrd[:, 0:Tg, :].rearrange("p t (h c) -> p t h c", h=2, c=C)
                    nc.vector.tensor_tensor(
                        out=s4, in0=s4,
                        in1=att_rep_t[:].rearrange("p (o h c) -> p o h c", o=1, h=2, c=C)
                            .to_broadcast([P, Tg, 2, C]),
                        op=OP.mult)
                    # e[p, t, h] = sum_c w
                    e2 = ep.tile([P, 2 * TG_MAX], F16, tag="e2")
                    e23 = e2[:, 0:2 * Tg].rearrange("p (t h) -> p t h", t=Tg, h=2)
                    with nc.allow_low_precision(reason="fp16 e-scores"):
                        nc.vector.tensor_reduce(
                            out=e23, in_=s4, axis=mybir.AxisListType.X, op=OP.add)
                    ee = ep.tile([P, 2 * TG_MAX], F16, tag="ee")
                    nc.scalar.activation(ee[:, 0:2 * Tg], e2[:, 0:2 * Tg], AF.Exp)
                    ee3 = ee[:, 0:2 * Tg].rearrange("p (t h) -> p t h", t=Tg, h=2)
                    # pay = [ee*xl | ee]
                    pay = gp.tile([P, TG_MAX, HC + 2], F16, tag="pay")
                    nc.vector.tensor_tensor(
                        out=pay[:, 0:Tg, 0:HC].rearrange("p t (h c) -> p t h c", h=2, c=C),
                        in0=xls[:, 0:Tg, :].rearrange("p t (h c) -> p t h c", h=2, c=C),
                        in1=ee3.unsqueeze(3).to_broadcast([P, Tg, 2, C]),
                        op=OP.mult)
                    nc.vector.tensor_copy(pay[:, 0:Tg, HC:HC + 2], ee3)
                    # one-hot dst
                    oh = gp.tile([P, TG_MAX, P], F16, tag="oh")
                    nc.vector.tensor_tensor(
                        out=oh[:, 0:Tg, :],
                        in0=iota_h[:].rearrange("p (o c) -> p o c", o=1, c=P)
                            .to_broadcast([P, Tg, P]),
                        in1=edstl_sb[:, c0:c0 + Tg].unsqueeze(2).to_broadcast([P, Tg, P]),
                        op=OP.is_equal)

                    for b in bs:
                        cols = (list(range(lo_col[b] - c0, lo_col[b] - c0 + T_lo[b]))
                                + list(range(hi_col[b] - c0, hi_col[b] - c0 + T_hi[b])))
                        acc = ps_acc.tile([P, HC + 2], F32, space="PSUM", tag="acc")
                        for i, t in enumerate(cols):
                            nc.tensor.matmul(acc[:], lhsT=oh[:, t, :], rhs=pay[:, t, :],
                                             start=(i == 0), stop=(i == len(cols) - 1))
                        # ---- drain block b ----
                        last = b == BLOCKS - 1
                        d2 = sb.tile([P, 2], F32, tag="d2")
                        nc.scalar.activation(d2[:], acc[:, HC:HC + 2], AF.Copy,
                                             bias=1e-20, scale=2.0)
                        rec = sb.tile([P, 2], F32, tag="rec")
                        nc.vector.reciprocal(rec[:], d2[:])
                        hs = sb.tile([P, C + 1], F32, tag="hs")
                        nc.vector.memset(hs[:, C:C + 1], 1.0)
                        nc.vector.tensor_scalar_mul(hs[:, 0:C], acc[:, 0:C], rec[:, 0:1])
                        nc.vector.scalar_tensor_tensor(
                            out=hs[:, 0:C], in0=acc[:, C:HC], scalar=rec[:, 1:2],
                            in1=hs[:, 0:C], op0=OP.mult, op1=OP.add)
                        nc.vector.tensor_add(hs[:, 0:C], hs[:, 0:C], B_rep[:])
                        if last and LAST < P:
                            nc.vector.tensor_scalar_mul(hs[:], hs[:], mask_col[:, 0:1])
                        nc.tensor.matmul(pstats[:], lhsT=hs[:, 0:C], rhs=hs[:],
                                         start=(b == 0), stop=(b == BLOCKS - 1))
                        pht = ps_t.tile([C, P], F32, space="PSUM", tag="ps_tr")
                        nc.tensor.transpose(pht[:], hs[:, 0:C], ident[:])
                        nc.scalar.copy(hT_res[:, b * P:(b + 1) * P], pht[:])

                # ---- stats finalize + AllReduce ----
                trash = sb.tile([C, C], F32, tag="trash")
                st2 = sb.tile([C, 2], F32, tag="st2")
                nc.vector.tensor_mul(trash[:], pstats[:, 0:C], ident[0:C, 0:C])
                nc.vector.tensor_reduce(st2[:, 1:2], trash[:], axis=mybir.AxisListType.X, op=OP.add)
                nc.vector.tensor_copy(st2[:, 0:1], pstats[:, C:C + 1])
                nc.sync.dma_start(out=st_l[li - 1][:], in_=st2[:])
                nc.gpsimd.collective_compute(
                    "AllReduce", OP.add, replica_groups=rg,
                    ins=[st_l[li - 1][:]], outs=[st_g[li - 1][:]])
                stg_t = sb.tile([C, 2], F32, tag="stg")
                nc.sync.dma_start(out=stg_t[:], in_=st_g[li - 1][:])
                # A = gng * rsqrt(var+eps); B = gnb - A*a*mean
                a_col = load_col(w[f"gna{li}"], C)
                g_col = load_col(w[f"gng{li}"], C)
                bta_col = load_col(w[f"gnb{li}"], C)
                mean = sb.tile([C, 1], F32, tag="gn_m")
                nc.scalar.activation(mean[:], stg_t[:, 0:1], AF.Copy, bias=0.0, scale=1.0 / cfg.N)
                msq = sb.tile([C, 1], F32, tag="gn_m2")
                nc.scalar.square(msq[:], mean[:])
                qn = sb.tile([C, 1], F32, tag="gn_qn")
                nc.scalar.activation(qn[:], stg_t[:, 1:2], AF.Copy, bias=0.0, scale=1.0 / cfg.N)
                a2 = sb.tile([C, 1], F32, tag="gn_a2")
                nc.vector.tensor_mul(a2[:], a_col[:], a_col[:])
                twoa = sb.tile([C, 1], F32, tag="gn_2a")
                nc.scalar.activation(twoa[:], a_col[:], AF.Copy, bias=0.0, scale=2.0)
                coef = sb.tile([C, 1], F32, tag="gn_cf")
                nc.vector.tensor_sub(coef[:], twoa[:], a2[:])
                cm = sb.tile([C, 1], F32, tag="gn_cm")
                nc.vector.tensor_mul(cm[:], coef[:], msq[:])
                var = sb.tile([C, 1], F32, tag="gn_var")
                nc.vector.tensor_sub(var[:], qn[:], cm[:])
                vare = sb.tile([C, 1], F32, tag="gn_vare")
                nc.vector.tensor_scalar_add(vare[:], var[:], cfg.EPS)
                lnv = sb.tile([C, 1], F32, tag="gn_lnv")
                nc.scalar.activation(lnv[:], vare[:], AF.Ln)
                rs = sb.tile([C, 1], F32, tag="gn_rs")
                nc.scalar.activation(rs[:], lnv[:], AF.Exp, bias=0.0, scale=-0.5)
                A = sb.tile([C, 1], F32, tag="gn_A")
                nc.vector.tensor_mul(A[:], g_col[:], rs[:])
                t_ = sb.tile([C, 1], F32, tag="gn_t")
                nc.vector.tensor_mul(t_[:], A[:], a_col[:])
                t2_ = sb.tile([C, 1], F32, tag="gn_t2")
                nc.vector.tensor_mul(t2_[:], t_[:], mean[:])
                B = sb.tile([C, 1], F32, tag="gn_B")
                nc.vector.tensor_sub(B[:], bta_col[:], t2_[:])
                return A, B

            A1, B1 = edge_layer(1, att_rep[0], B1_rep, h1T_res)

            # AllGather h1 (fp16, transposed layout)
            nc.sync.dma_start(out=h1T_dr[:], in_=h1T_res[:])
            nc.gpsimd.collective_compute(
                "AllGather", OP.bypass, replica_groups=rg,
                ins=[h1T_dr[:]], outs=[h1T_ag[:]])

            # ---------------- layer-2 folded weights ----------------
            Wl2p = con.tile([C, HC], F32, tag="Wl2p")
            nc.vector.tensor_scalar_mul(Wl2p[:], Wl2_sb[:], A1[:])
            Wl2ph = con.tile([C, HC], F16, tag="Wl2ph")
            nc.vector.tensor_copy(Wl2ph[:], Wl2p[:])
            Wr2p = con.tile([C, HC], F32, tag="Wr2p")
            nc.vector.tensor_scalar_mul(Wr2p[:], Wr2_sb[:], A1[:])
            Wr2ph = con.tile([C, HC], F16, tag="Wr2ph")
            nc.vector.tensor_copy(Wr2ph[:], Wr2p[:])

            def fold_bias(W_sb, b_row, n, tag):
                pb = ps_mm.tile([1, n], F32, space="PSUM", tag="ps_tab")
                nc.tensor.matmul(pb[:], lhsT=B1[:], rhs=W_sb[:], start=True, stop=True)
                brow = con.tile([1, n], F32, tag=f"brow_{tag}")
                nc.vector.tensor_add(brow[:], pb[:], b_row[:])
                return brow

            bl2p_row = fold_bias(Wl2_sb, bl2_row, HC, "l2l")
            br2p_row = fold_bias(Wr2_sb, br2_row, HC, "l2r")
            cb2_row = con.tile([1, HC], F32, tag="cb2_row")
            nc.vector.tensor_add(cb2_row[:], bl2p_row[:], br2p_row[:])
            cb2_rep = replicate_row(cb2_row[:], HC, "cb2_rep")
            # B2_rep = mean-head(bl2p) + bias2
            b2m = con.tile([1, C], F32, tag="b2m")
            nc.vector.tensor_add(b2m[:], bl2p_row[:, 0:C], bl2p_row[:, C:HC])
            nc.vector.tensor_scalar_mul(b2m[:], b2m[:], 0.5)
            nc.vector.tensor_add(b2m[:], b2m[:], bias2_row[:])
            B2_rep = replicate_row(b2m[:], C, "B2_rep")

            # ---------------- layer-2 tables ----------------
            def get_h1_chunk(ch):
                t = sb.tile([C, CHUNK * P], F16, tag="h1_chunk")
                j0 = ch * CHUNK
                # merge maximal same-core runs into single DMAs
                j = 0
                while j < CHUNK:
                    k, b = divmod(j0 + j, BLOCKS)
                    run = 1
                    while j + run < CHUNK and (j0 + j + run) // BLOCKS == k:
                        run += 1
                    nc.sync.dma_start(
                        out=t[:, j * P:(j + run) * P],
                        in_=h1T_ag[k * C:(k + 1) * C, b * P:(b + run) * P])
                    j += run
                return [t[:, j * P:(j + 1) * P] for j in range(CHUNK)]

            build_xl_table(2, get_h1_chunk, Wl2ph)
            build_xr_table(2, lambda b: h1T_res[:, b * P:(b + 1) * P], Wr2ph, cb2_rep)

            A2, B2gn = edge_layer(2, att_rep[1], B2_rep, h2T_res)

            # ---------------- classifier + log_softmax ----------------
            W1pf = con.tile([C, NCLS], F32, tag="W1pf")
            nc.vector.tensor_scalar_mul(W1pf[:], W1_sb[:], A2[:])
            W1p = con.tile([C, NCLS], F16, tag="W1p")
            nc.vector.tensor_copy(W1p[:], W1pf[:])
            pb1 = ps_mm.tile([1, NCLS], F32, space="PSUM", tag="ps_tab")
            nc.tensor.matmul(pb1[:], lhsT=B2gn[:], rhs=W1_sb[:], start=True, stop=True)
            b1p = con.tile([1, NCLS], F32, tag="b1p")
            nc.vector.tensor_add(b1p[:], pb1[:], b1_row[:])
            b1p_rep = replicate_row(b1p[:], NCLS, "b1p_rep")

            for b in range(BLOCKS):
                pl = ps_acc.tile([P, NCLS], F32, space="PSUM", tag="acc")
                nc.tensor.matmul(pl[:], lhsT=h2T_res[:, b * P:(b + 1) * P], rhs=W1p[:],
                                 start=True, stop=True)
                lg = sb.tile([P, NCLS], F32, tag="lg")
                nc.vector.tensor_add(lg[:], pl[:], b1p_rep[:])
                mx = sb.tile([P, 1], F32, tag="mx")
                nc.vector.tensor_reduce(mx[:], lg[:], axis=mybir.AxisListType.X, op=OP.max)
                lgm = sb.tile([P, NCLS], F32, tag="lgm")
                nc.vector.tensor_scalar(out=lgm[:], in0=lg[:], scalar1=mx[:, 0:1],
                                        scalar2=None, op0=OP.subtract)
                ex = sb.tile([P, NCLS], F32, tag="ex")
                nc.scalar.activation(ex[:], lgm[:], AF.Exp)
                sm = sb.tile([P, 1], F32, tag="sm")
                nc.vector.tensor_reduce(sm[:], ex[:], axis=mybir.AxisListType.X, op=OP.add)
                lns = sb.tile([P, 1], F32, tag="lns")
                nc.scalar.activation(lns[:], sm[:], AF.Ln)
                ot = sb.tile([P, NCLS], F32, tag="ot")
                nc.vector.tensor_scalar(out=ot[:], in0=lgm[:], scalar1=lns[:, 0:1],
                                        scalar2=None, op0=OP.subtract)
                rows = min(P, NPC - b * P)
                nc.sync.dma_start(out=out_d[b * P: b * P + rows, :], in_=ot[0:rows, :])

    nc.compile()
    return nc


_CACHE = {}


def _get_program(cfg, sched):
    key = (tuple(sched["T_lo"]), tuple(sched["T_hi"]))
    if key not in _CACHE:
        _CACHE[key] = _build(cfg, sched)
    return _CACHE[key]


def _install_axon_ntff_shim():
    """Provide antenv.axon_hooks (missing on this image) so trace=True works."""
    import sys, types, ctypes, contextlib, glob as _glob
    try:
        import antenv.axon_hooks  # noqa
        return
    except ImportError:
        pass
    hook = None
    for so_path in (["/opt/axon/libaxon_pjrt.so"] + _glob.glob("/root/.axon_site/**/libaxon_pjrt.so", recursive=True)):
        try:
            lib = ctypes.CDLL(so_path)
        except OSError:
            continue
        if not hasattr(lib, "axon_start_nrt_profile"):
            continue
        lib.axon_start_nrt_profile.argtypes = [ctypes.POINTER(ctypes.c_int64), ctypes.c_size_t]
        lib.axon_start_nrt_profile.restype = ctypes.c_int64
        lib.axon_stop_nrt_profile.argtypes = [ctypes.c_char_p]
        lib.axon_stop_nrt_profile.restype = ctypes.c_int64

        @contextlib.contextmanager
        def _hook(output_dir, device_ids, _lib=lib):
            import jax
            jax.devices()
            if device_ids:
                ids = (ctypes.c_int64 * len(device_ids))(*device_ids)
                rc = _lib.axon_start_nrt_profile(ids, len(device_ids))
            else:
                rc = _lib.axon_start_nrt_profile(None, 0)
            if rc != 0:
                raise RuntimeError(f"axon_start_nrt_profile rc={rc}")
            try:
                yield
            finally:
                n = _lib.axon_stop_nrt_profile(str(output_dir).encode())
                print(f"ntff profile: {n} file(s) -> {output_dir}")

        hook = _hook
        break
    m = types.ModuleType("antenv.axon_hooks")
    m.get_axon_ntff_profile_hook = lambda: hook
    m.set_axon_ntff_profile_hook = lambda h: None
    sys.modules["antenv.axon_hooks"] = m
    try:
        import antenv
        antenv.axon_hooks = m
    except ImportError:
        pass
    import concourse.bass_utils as bu
    bu.upload_artifacts = lambda tmpdir: str(tmpdir)


def kernel(**inputs):
    from concourse.bass_utils import run_bass_kernel_spmd
    import os

    x = np.ascontiguousarray(np.asarray(inputs["x"], dtype=np.float32))
    edge_index = np.asarray(inputs["edge_index"], dtype=np.int32)
    cfg = Cfg(x.shape[0], 8)
    sched, per_core = _preprocess(cfg, x.astype(np.float16), edge_index)
    nc = _get_program(cfg, sched)

    f32 = lambda a: np.ascontiguousarray(np.asarray(a, dtype=np.float32))
    f16 = lambda a: np.ascontiguousarray(np.asarray(a, dtype=np.float32).astype(np.float16))

    bl1 = f32(inputs["bl1"]).reshape(-1)
    br1 = f32(inputs["br1"]).reshape(-1)
    bias1 = f32(inputs["bias1"]).reshape(-1)
    att1 = f32(inputs["att1"])    # [2, 64]
    att2 = f32(inputs["att2"])
    C = cfg.C

    mask = np.zeros((P, 1), np.float32)
    lastn = cfg.NPC - (cfg.BLOCKS - 1) * P
    mask[:lastn] = 1.0

    base = {
        "Wl1h": f16(inputs["Wl1"]),
        "Wr1h": f16(inputs["Wr1"]),
        "cb1_rep": np.ascontiguousarray(np.broadcast_to(bl1 + br1, (P, cfg.HC)).astype(np.float32)),
        "B1_rep": np.ascontiguousarray(
            np.broadcast_to((bl1[0:C] + bl1[C:2 * C]) * 0.5 + bias1, (P, C)).astype(np.float32)),
        "att1_rep": np.ascontiguousarray(np.broadcast_to(att1.reshape(-1), (P, cfg.HC)).astype(np.float16)),
        "att2_rep": np.ascontiguousarray(np.broadcast_to(att2.reshape(-1), (P, cfg.HC)).astype(np.float16)),
        "maskcol": mask,
        "Wl2": f32(inputs["Wl2"]),
        "Wr2": f32(inputs["Wr2"]),
        "bl2": f32(inputs["bl2"]).reshape(-1),
        "br2": f32(inputs["br2"]).reshape(-1),
        "bias2": f32(inputs["bias2"]).reshape(-1),
        "gng1": f32(inputs["gng1"]).reshape(-1), "gnb1": f32(inputs["gnb1"]).reshape(-1),
        "gna1": f32(inputs["gna1"]).reshape(-1),
        "gng2": f32(inputs["gng2"]).reshape(-1), "gnb2": f32(inputs["gnb2"]).reshape(-1),
        "gna2": f32(inputs["gna2"]).reshape(-1),
        "W1": f32(inputs["W1"]),
        "b1": f32(inputs["b1"]).reshape(-1),
    }
    in_maps = [{**base, **pc} for pc in per_core]

    trace = bool(int(os.environ.get("GAT_TRACE", "0")))
    if trace:
        _install_axon_ntff_shim()
    r = run_bass_kernel_spmd(nc, in_maps, core_ids=list(range(cfg.NC)), trace=trace)
    kernel.last_results = r
    if trace and r.exec_time_ns is not None:
        print(f"HW exec time: {r.exec_time_ns} ns")
        if r.instructions_and_trace is not None:
            print(f"trace: {r.instructions_and_trace[1]}")
        print(f"profile_json: {r.profile_json}")
        kernel.last_exec_ns = r.exec_time_ns
    out = np.concatenate([r.results[k]["out"] for k in range(cfg.NC)], axis=0)
    return out
